# revision 25
# baseline (speedup 1.0000x reference)
"""Trainium2 Bass kernel for nn_AssignModule segment_reduce (voxel seeding).

Pipeline (mirrors the reference exactly):
  1. per-component bbox (segment min/max)         -> device (graph A)
  2. 8-step bisection on voxel size v; per-point grid coords
     floor((x-mn)/v) with f32-exact-division floor semantics -> device
     (graph B; boundary-"suspect" points, ~1e-4 of all, are flagged on
     device and recomputed exactly host-side), per-component distinct-key
     counts + O(G)=64 bracket updates on host
  3. final voxelization: unique keys / centroid / argmin assembly (host)

Sharding: comp_lid is sorted so components partition cleanly; core c takes
components [8c, 8c+8).  Inside a core, every SBUF partition holds points of
exactly one component (component ranges padded to the per-partition width),
so per-component scalars (bbox mins, 1/v, suspect threshold) are
per-partition scalars.

Graph B output: 3x int16 [128, W]; g0 carries the suspect flag in its sign
(g0 - 32768*flag).  Graphs are compiled once and executed 1x (A) + 9x (B).
"""

import numpy as np

N = 2097152
G = 64
N_BISECT = 8
N_CORES = 8
COMPS_PER_CORE = G // N_CORES
P1, P2, P3 = 73856093, 19349663, 83492791
MASK40 = (1 << 40) - 1
TWO23 = float(2 << 22)

LAST_DEBUG = {}


# ----------------------------------------------------------------------------
# device graphs
# ----------------------------------------------------------------------------

def build_graph_bbox(W):
    """Per-partition bbox min/max, 2 chunks to overlap DMA with reduces.
    mm output: [128, 12] = two chunks x (3 min, 3 max); host combines."""
    from concourse import bass, mybir

    nc = bass.Bass(trn_type="TRN2")
    dt = mybir.dt
    op = mybir.AluOpType

    pts_ext = nc.declare_dram_parameter("pts", [128, 3 * W], dt.float32, isOutput=False)
    mm_ext = nc.declare_dram_parameter("mm", [128, 24], dt.float32, isOutput=True)

    cuts = [0, W // 4, W // 2, 3 * W // 4, W]
    chunks = list(zip(cuts[:-1], cuts[1:]))

    with (
        nc.Block() as block,
        nc.semaphore("dma") as dma,
        nc.semaphore("cmp") as cmp,
        nc.sbuf_tensor("xyz", [128, 3 * W], dt.float32) as xyz,
        nc.sbuf_tensor("mm_sb", [128, 24], dt.float32) as mm,
    ):
        @block.sync
        def _(sy):
            for a, b in chunks:
                sy.dma_start(
                    out=xyz[:, 3 * a : 3 * b], in_=pts_ext[:, 3 * a : 3 * b]
                ).then_inc(dma, 16)
            sy.wait_ge(cmp, 4)
            sy.dma_start(out=mm_ext[:, :], in_=mm[:, :]).then_inc(dma, 16)
            sy.wait_ge(dma, 80)

        @block.vector
        def _(v):
            for k, (a, b) in enumerate(chunks):
                v.wait_ge(dma, 16 * (k + 1))
                for c in range(3):
                    xv = xyz[:, 3 * a + c : 3 * b : 3]
                    o = 6 * k
                    v.tensor_reduce(
                        mm[:, o + c : o + c + 1], xv, mybir.AxisListType.XYZW, op.min
                    )
                    ins = v.tensor_reduce(
                        mm[:, o + 3 + c : o + 4 + c], xv, mybir.AxisListType.XYZW,
                        op.max,
                    )
                    if c == 2:
                        ins.then_inc(cmp, 1)

    return nc


def build_graph_coord(W, out_dt_name="int16"):
    """Per-point grid coords + suspect flag.

    Chunk-pipelined over two engines: ACT computes d = x-mn, q0 = d*(1/v),
    r = round_to_i16(q0) for chunk k while DVE runs the floor fix-up
    (fl = r - (r > q0)), the two-sided boundary-suspect test, and the
    flag pack on chunk k-1.  (GPSIMD was evaluated as a third worker but
    the Pool engine lacks comparison ALU ops.)

    scal columns: 0..2 = -mins(xyz), 3 = 1/v, 4 = Tlo (2^-21*qmax),
    5 = Thi (1 - Tlo), 6 = 2^23, 7 = -2^23.
    """
    from concourse import bass, mybir

    nc = bass.Bass(trn_type="TRN2")
    dt = mybir.dt
    op = mybir.AluOpType
    af = mybir.ActivationFunctionType
    odt = getattr(dt, out_dt_name)
    packc = -32768.0 if out_dt_name == "int16" else -float(2**30)

    pts_ext = nc.declare_dram_parameter("pts", [128, 3 * W], dt.float32, isOutput=False)
    scal_ext = nc.declare_dram_parameter("scal", [128, 8], dt.float32, isOutput=False)
    g_ext = [
        nc.declare_dram_parameter(f"g{c}", [128, W], odt, isOutput=True)
        for c in range(3)
    ]

    # eighths; all chunks on the vector engine (Pool lacks compare ops)
    fr8 = [0, 1, 3, 6, 8]
    owners = ["D", "D", "D", "D"]
    cuts = [W * f // 8 for f in fr8]
    chunks = list(zip(cuts[:-1], cuts[1:]))
    NCH = len(chunks)

    ctx = []

    def sb(name, shape, dtype):
        cm = nc.sbuf_tensor(name, shape, dtype)
        h = cm.__enter__()
        ctx.append(cm)
        return h

    Wq = max(b - a for a, b in chunks)
    xyz = sb("xyz", [128, 3 * W], dt.float32)
    scal = sb("scal_sb", [128, 8], dt.float32)
    bd = sb("bd", [128, Wq], dt.float32)
    q0 = [sb(f"q0_{c}", [128, W], dt.float32) for c in range(3)]
    rr = [sb(f"rr_{c}", [128, W], dt.int16) for c in range(3)]
    gi = [sb(f"gi{c}", [128, W], odt) for c in range(3)]
    # chunk-local temp sets per worker engine
    tmp = {}
    for e in ("D",):
        tmp[e] = dict(
            bC=sb(f"bC{e}", [128, Wq], dt.float32),
            bD=sb(f"bD{e}", [128, Wq], dt.float32),
            ss=[sb(f"ss{e}{c}", [128, Wq], dt.float32) for c in range(3)],
            fl0=sb(f"fl0{e}", [128, Wq], odt),
        )

    def worker_body(v, e, done_sem, cmpA):
        t = tmp[e]
        bC, bD, ss, fl0 = (t["bC"], t["bD"], t["ss"], t["fl0"])
        mt2 = scal[:, 4:5]
        v.wait_ge(cmpA, 1)
        for k, (a, b) in enumerate(chunks):
            if owners[k] != e:
                continue
            v.wait_ge(cmpA, k + 2)
            s = slice(a, b)
            q = slice(0, b - a)
            if True:
                # DVE: mixed int16/f32 allowed; fl lands in int16 outputs
                for c in range(3):
                    dst, ds = (fl0, q) if c == 0 else (gi[c], s)
                    v.tensor_tensor(bC[:, q], rr[c][:, s], q0[c][:, s], op.is_gt)
                    v.tensor_tensor(dst[:, ds], rr[c][:, s], bC[:, q], op.subtract)
                    # fr = q0 - fl (exact); m~ = fr^2 - fr is near 0 iff fr
                    # is near 0 or 1 (the suspect zone)
                    v.tensor_tensor(bD[:, q], q0[c][:, s], dst[:, ds], op.subtract)
                    v.scalar_tensor_tensor(
                        ss[c][:, q], bD[:, q], 1.0, bD[:, q], op.subtract, op.mult
                    )
                v.tensor_tensor(bC[:, q], ss[0][:, q], ss[1][:, q], op.max)
                v.tensor_tensor(bD[:, q], bC[:, q], ss[2][:, q], op.max)
                v.tensor_scalar(bC[:, q], bD[:, q], mt2, packc, op.is_gt, op.mult)
                v.tensor_tensor(gi[0][:, s], fl0[:, q], bC[:, q], op.add).then_inc(
                    done_sem, 1
                )

    with (
        nc.Block() as block,
        nc.semaphore("din") as din,
        nc.semaphore("dout") as dout,
        nc.semaphore("cmpA") as cmpA,
        nc.semaphore("cmpD") as cmpD,
        nc.semaphore("cmpG") as cmpG,
    ):
        @block.sync
        def _(sy):
            sy.dma_start(out=scal[:, :], in_=scal_ext[:, :]).then_inc(din, 16)
            for a, b in chunks:
                sy.dma_start(
                    out=xyz[:, 3 * a : 3 * b], in_=pts_ext[:, 3 * a : 3 * b]
                ).then_inc(din, 16)
            nd = ng = 0
            for k in range(NCH):
                a, b = chunks[k]
                if owners[k] == "D":
                    nd += 1
                    sy.wait_ge(cmpD, nd)
                else:
                    ng += 1
                    sy.wait_ge(cmpG, ng)
                for c in range(3):
                    sy.dma_start(
                        out=g_ext[c][:, a:b], in_=gi[c][:, a:b]
                    ).then_inc(dout, 16)
            sy.wait_ge(dout, NCH * 48)

        @block.scalar
        def _(sc):
            negmn = [scal[:, c : c + 1] for c in range(3)]
            rc = scal[:, 3:4]
            sc.activation(bd[:, 0:4], bd[:, 0:4], af.Copy).then_inc(cmpA, 1)
            for k, (a, b) in enumerate(chunks):
                sc.wait_ge(din, 32 + 16 * k)
                s = slice(a, b)
                q = slice(0, b - a)
                for c in range(3):
                    xv = xyz[:, 3 * a + c : 3 * b : 3]
                    sc.activation(bd[:, q], xv, af.Identity, bias=negmn[c], scale=1.0)
                    sc.activation(q0[c][:, s], bd[:, q], af.Copy, bias=0.0, scale=rc)
                    ins = sc.activation(rr[c][:, s], q0[c][:, s], af.Copy)
                    if c == 2:
                        ins.then_inc(cmpA, 1)

        @block.vector
        def _(v):
            worker_body(v, "D", cmpD, cmpA)

    for cm in reversed(ctx):
        cm.__exit__(None, None, None)
    return nc


# ----------------------------------------------------------------------------
# compile-once SPMD runner (mirrors bass2jax.run_bass_via_pjrt)
# ----------------------------------------------------------------------------

class SpmdRunner:
    def __init__(self, nc, n_cores=N_CORES):
        import jax
        from concourse import mybir
        from concourse.bass2jax import (
            _bass_exec_p,
            install_neuronx_cc_hook,
            partition_id_tensor,
        )
        from jax.sharding import Mesh, PartitionSpec
        try:
            from jax.experimental.shard_map import shard_map
        except ImportError:
            from jax.shard_map import shard_map

        install_neuronx_cc_hook()
        assert not (nc.dbg_addr is not None and nc.dbg_callbacks)
        self.nc = nc
        self.n_cores = n_cores
        partition_name = nc.partition_id_tensor.name if nc.partition_id_tensor else None

        in_names, out_names, out_avals, zero_shapes = [], [], [], []
        for alloc in nc.m.functions[0].allocations:
            if not isinstance(alloc, mybir.MemoryLocationSet):
                continue
            name = alloc.memorylocations[0].name
            if alloc.kind == "ExternalInput":
                if name != partition_name and name != (
                    nc.dbg_addr.name if nc.dbg_addr is not None else None
                ):
                    in_names.append(name)
            elif alloc.kind == "ExternalOutput":
                shape = tuple(alloc.tensor_shape)
                dtype = mybir.dt.np(alloc.dtype)
                out_names.append(name)
                out_avals.append(jax.core.ShapedArray(shape, dtype))
                zero_shapes.append((shape, dtype))
        self.has_dbg = nc.dbg_addr is not None
        if self.has_dbg:
            in_names.append(nc.dbg_addr.name)
        n_params = len(in_names)
        self.in_names = list(in_names)
        self.out_names = out_names
        self.out_avals = out_avals
        self.zero_shapes = zero_shapes
        self.n_params = n_params

        all_in_names = in_names + out_names
        if partition_name is not None:
            all_in_names.append(partition_name)
        donate = tuple(range(n_params, n_params + len(out_names)))

        def _body(*args):
            operands = list(args)
            if partition_name is not None:
                operands.append(partition_id_tensor())
            outs = _bass_exec_p.bind(
                *operands,
                out_avals=tuple(out_avals),
                in_names=tuple(all_in_names),
                out_names=tuple(out_names),
                lowering_input_output_aliases=(),
                sim_require_finite=True,
                sim_require_nnan=True,
                nc=nc,
            )
            return tuple(outs)

        devices = jax.devices()[:n_cores]
        mesh = Mesh(np.asarray(devices), ("core",))
        in_specs = (PartitionSpec("core"),) * (n_params + len(out_names))
        out_specs = (PartitionSpec("core"),) * len(out_names)
        self._fn = jax.jit(
            shard_map(
                _body,
                mesh=mesh,
                in_specs=in_specs,
                out_specs=out_specs,
                check_rep=False,
            ),
            donate_argnums=donate,
            keep_unused=True,
        )

    def run(self, in_maps):
        per_core = []
        for m in in_maps:
            m = dict(m)
            if self.has_dbg:
                m[self.nc.dbg_addr.name] = np.zeros((1, 2), np.uint32)
            per_core.append([np.ascontiguousarray(m[k]) for k in self.in_names])
        concat_in = [
            np.concatenate([per_core[c][i] for c in range(self.n_cores)], axis=0)
            for i in range(self.n_params)
        ]
        concat_zeros = [
            np.zeros((self.n_cores * s[0], *s[1:]), d) for (s, d) in self.zero_shapes
        ]
        outs = self._fn(*concat_in, *concat_zeros)
        return [
            {
                name: np.asarray(outs[i]).reshape(
                    self.n_cores, *self.out_avals[i].shape
                )[c]
                for i, name in enumerate(self.out_names)
            }
            for c in range(self.n_cores)
        ]


# ----------------------------------------------------------------------------
# host orchestration
# ----------------------------------------------------------------------------

def _layout(comp_lid):
    """Shard points by component; pad each component to a multiple of the
    per-partition width W so each SBUF partition is single-component."""
    bounds = np.searchsorted(comp_lid, np.arange(G + 1)).astype(np.int64)
    lens = np.diff(bounds)
    core_pts = [
        int(bounds[(c + 1) * COMPS_PER_CORE] - bounds[c * COMPS_PER_CORE])
        for c in range(N_CORES)
    ]
    W = -(-max(core_pts) // 120)
    W = (W + 3) & ~3
    while True:
        ok = all(
            sum(
                -(-int(lens[g]) // W)
                for g in range(c * COMPS_PER_CORE, (c + 1) * COMPS_PER_CORE)
            )
            <= 128
            for c in range(N_CORES)
        )
        if ok:
            break
        W += 64
    slots = {}  # comp -> (core, part_start, n_parts, length)
    part2comp = np.zeros((N_CORES, 128), np.int64)
    for c in range(N_CORES):
        p = 0
        for g in range(c * COMPS_PER_CORE, (c + 1) * COMPS_PER_CORE):
            L = int(lens[g])
            k = -(-L // W)
            slots[g] = (c, p, k, L)
            part2comp[c, p : p + k] = g
            p += k
        part2comp[c, p:] = (c + 1) * COMPS_PER_CORE - 1
    return bounds, lens, W, slots, part2comp


def _pack_points(p_xyz, bounds, W, slots):
    pts = np.zeros((N_CORES, 128 * W, 3), np.float32)
    for g, (c, p, k, L) in slots.items():
        seg = p_xyz[bounds[g] : bounds[g + 1]]
        blk = pts[c, p * W : (p + k) * W]
        blk[:L] = seg
        blk[L:] = seg[0]
        if p + k < 128 and g % COMPS_PER_CORE == COMPS_PER_CORE - 1:
            pts[c, (p + k) * W :] = seg[0]
    return [pts[c].reshape(128, 3 * W) for c in range(N_CORES)]


def _unpack(outs, key, bounds, W, slots, dtype):
    full = np.empty(N, dtype)
    for g, (c, p, k, L) in slots.items():
        flat = outs[c][key].reshape(-1)
        full[bounds[g] : bounds[g + 1]] = flat[p * W : p * W + L]
    return full


def _grid_exact_host(x, mn, v):
    """floor of correctly-rounded f32 division, matching jax CPU."""
    d = (x.astype(np.float32) - mn.astype(np.float32)).astype(np.float32)
    q = (d / v.astype(np.float32)).astype(np.float32)
    return np.floor(q).astype(np.int64)


def kernel(p_xyz, comp_lid, s_alloc):
    p_xyz = np.asarray(p_xyz, np.float32)
    comp_lid = np.asarray(comp_lid, np.int64)
    s_alloc = np.asarray(s_alloc, np.int64)

    bounds, lens, W, slots, part2comp = _layout(comp_lid)
    pts_maps = _pack_points(p_xyz, bounds, W, slots)

    nc_bbox = build_graph_bbox(W)
    runner_bbox = SpmdRunner(nc_bbox)
    nc_coord = build_graph_coord(W)
    runner_coord = SpmdRunner(nc_coord)
    LAST_DEBUG.clear()
    LAST_DEBUG.update(
        nc_bbox=nc_bbox, nc_coord=nc_coord, W=W, n_exec=0, passes=[]
    )
    state = {"runner32": None}

    # ---- pass 0: bbox ----
    outs0 = runner_bbox.run([{"pts": pts_maps[c]} for c in range(N_CORES)])
    LAST_DEBUG["n_exec"] += 1
    mins = np.full((G, 3), np.inf, np.float32)
    maxs = np.full((G, 3), -np.inf, np.float32)
    for g, (c, p, k, L) in slots.items():
        rows = outs0[c]["mm"][p : p + k]
        rmin = rows[:, 0:3]
        rmax = rows[:, 3:6]
        for o in range(6, 24, 6):
            rmin = np.minimum(rmin, rows[:, o : o + 3])
            rmax = np.maximum(rmax, rows[:, o + 3 : o + 6])
        mins[g] = rmin.min(axis=0)
        maxs[g] = rmax.max(axis=0)
    LAST_DEBUG["mins"] = mins.copy()
    LAST_DEBUG["maxs"] = maxs.copy()

    f32 = np.float32
    span_true = (maxs - mins).astype(f32)  # for qmax bound

    def exec_pass(v_f32):
        """One coord pass; returns exact per-point int64 grids [N,3]."""
        qmax = span_true.max(axis=1) / v_f32 + 2.0  # float64 upper bound
        use16 = bool(np.all(qmax < 32000.0))
        if use16:
            runner = runner_coord
        else:
            if state["runner32"] is None:
                nc32 = build_graph_coord(W, "int32")
                state["runner32"] = SpmdRunner(nc32)
            runner = state["runner32"]
        scal_maps = []
        for c in range(N_CORES):
            sc = np.zeros((128, 8), np.float32)
            gidx = part2comp[c]
            sc[:, 0:3] = -mins[gidx]
            sc[:, 3] = (f32(1.0) / v_f32[gidx]).astype(f32)
            tp = (qmax[gidx] * (2.0**-20)).astype(f32)  # 2x-inflated threshold
            sc[:, 4] = -tp
            scal_maps.append(sc)
        outs = runner.run(
            [{"pts": pts_maps[c], "scal": scal_maps[c]} for c in range(N_CORES)]
        )
        LAST_DEBUG["n_exec"] += 1
        grids = np.empty((N, 3), np.int64)
        packc = 32768 if use16 else 2**30
        p0 = _unpack(outs, "g0", bounds, W, slots, np.int64)
        sus_mask = p0 < 0
        grids[:, 0] = np.where(sus_mask, p0 + packc, p0)
        grids[:, 1] = _unpack(outs, "g1", bounds, W, slots, np.int64)
        grids[:, 2] = _unpack(outs, "g2", bounds, W, slots, np.int64)
        sus = np.nonzero(sus_mask)[0]
        if sus.size:
            comp_s = comp_lid[sus]
            for axis in range(3):
                grids[sus, axis] = _grid_exact_host(
                    p_xyz[sus, axis], mins[comp_s, axis], v_f32[comp_s]
                )
        LAST_DEBUG["passes"].append(dict(v=v_f32.copy(), n_suspect=int(sus.size)))
        return grids

    # ---- host O(G) setup (mirrors reference f32/int64 semantics) ----
    span = np.maximum(span_true, f32(1e-6))
    safe_span = np.maximum(span, f32(0.05))
    safe_vol = (safe_span[:, 0] * safe_span[:, 1] * safe_span[:, 2]).astype(f32)
    n_per = lens.astype(np.int64)
    s_c = np.clip(s_alloc, 1, np.maximum(n_per, np.int64(1)))
    v0 = np.power(
        (safe_vol / np.maximum(s_c.astype(f32), f32(1.0))).astype(f32), 1.0 / 3.0
    ).astype(f32)
    v_lo = np.maximum((v0 * f32(0.1)).astype(f32), f32(1e-4))
    v_hi = np.maximum((v0 * f32(64.0)).astype(f32), f32(1e-4))
    best_v = v0.copy()
    best_diff = np.full(G, 1 << 30, np.int64)

    def distinct_counts(grids):
        h = (
            (grids[:, 0] * np.int64(P1))
            ^ (grids[:, 1] * np.int64(P2))
            ^ (grids[:, 2] * np.int64(P3))
        ) & np.int64(MASK40)
        cnt = np.empty(G, np.int64)
        for g in range(G):
            cnt[g] = np.unique(h[bounds[g] : bounds[g + 1]]).size
        return cnt, h

    # ---- bisection ----
    # best_v is always one of the visited v_mids (first iteration always
    # improves the 1<<30 sentinel), so cache per-pass grids and skip the
    # final voxelization pass entirely.
    pass_grids = []
    best_pass = np.zeros(G, np.int64)
    for it in range(N_BISECT):
        v_mid = ((v_lo + v_hi) * f32(0.5)).astype(f32)
        grids = exec_pass(v_mid)
        pass_grids.append(grids.astype(np.int32))
        cnt, _ = distinct_counts(grids)
        curr_diff = np.abs(cnt - s_c)
        improved = curr_diff < best_diff
        best_diff = np.where(improved, curr_diff, best_diff)
        best_v = np.where(improved, v_mid, best_v).astype(f32)
        best_pass = np.where(improved, it, best_pass)
        more = cnt > s_c
        v_lo = np.where(more, v_mid, v_lo).astype(f32)
        v_hi = np.where(more, v_hi, v_mid).astype(f32)
        LAST_DEBUG["passes"][-1]["cnt"] = cnt.copy()

    # ---- final voxelization: reuse the winning pass's grids per component
    grids = np.empty((N, 3), np.int64)
    for g in range(G):
        sl = slice(bounds[g], bounds[g + 1])
        grids[sl] = pass_grids[int(best_pass[g])][sl]
    _, h = distinct_counts(grids)
    keys = (comp_lid << np.int64(40)) | h

    uniq, inv = np.unique(keys, return_inverse=True)
    U = uniq.size
    inv = inv.astype(np.int64)

    counts = np.bincount(inv, minlength=U).astype(np.int64)
    sums = np.zeros((U, 3), np.float32)
    np.add.at(sums, inv, p_xyz)
    cent = (sums / np.maximum(counts, 1).astype(np.float32)[:, None]).astype(
        np.float32
    )
    diff = (p_xyz - cent[inv]).astype(np.float32)
    d = (
        (diff[:, 0] * diff[:, 0] + diff[:, 1] * diff[:, 1]) + diff[:, 2] * diff[:, 2]
    ).astype(np.float32)

    order = np.argsort(inv, kind="stable")
    starts = np.searchsorted(inv[order], np.arange(U))
    dmin = np.minimum.reduceat(d[order], starts)
    pos = np.arange(N, dtype=np.int64)
    cand = np.where(d <= dmin[inv], pos, np.int64(N))
    argmin = np.minimum.reduceat(cand[order], starts)

    seeds_xyz = np.zeros((N, 3), np.float32)
    seeds_gid = np.full(N, -1, np.int64)
    valid = np.zeros(N, np.bool_)
    idx = np.clip(argmin, 0, N - 1)
    seeds_xyz[:U] = p_xyz[idx]
    seeds_gid[:U] = idx
    valid[:U] = True
    LAST_DEBUG["U"] = U
    return seeds_xyz, seeds_gid, valid


# revision 34
# speedup vs baseline: 1.2835x; 1.2835x over previous
"""Trainium2 Bass kernel for nn_AssignModule segment_reduce (voxel seeding).

Pipeline (mirrors the reference exactly):
  1. per-component bbox (segment min/max)         -> device (graph A)
  2. 8-step bisection on voxel size v; per-point grid coords
     floor((x-mn)/v) with f32-exact-division floor semantics -> device
     (graph B; boundary-"suspect" points, ~1e-4 of all, are flagged on
     device and recomputed exactly host-side), per-component distinct-key
     counts + O(G)=64 bracket updates on host
  3. final voxelization: unique keys / centroid / argmin assembly (host)

Sharding: comp_lid is sorted so components partition cleanly; core c takes
components [8c, 8c+8).  Inside a core, every SBUF partition holds points of
exactly one component (component ranges padded to the per-partition width),
so per-component scalars (bbox mins, 1/v, suspect threshold) are
per-partition scalars.

Graph B output: 3x int16 [128, W]; g0 carries the suspect flag in its sign
(g0 - 32768*flag).  Two NEFFs are compiled once and executed 9x total
(1x bbox + 8x bisection); the final voxelization reuses the winning
bisection pass's cached grids, since best_v is always one of the visited
v_mids.
"""

import numpy as np

N = 2097152
G = 64
N_BISECT = 8
N_CORES = 8
COMPS_PER_CORE = G // N_CORES
P1, P2, P3 = 73856093, 19349663, 83492791
MASK40 = (1 << 40) - 1
TWO23 = float(2 << 22)

LAST_DEBUG = {}


# ----------------------------------------------------------------------------
# device graphs
# ----------------------------------------------------------------------------

def build_graph_bbox(W):
    """Per-partition bbox min/max, 2 chunks to overlap DMA with reduces.
    mm output: [128, 12] = two chunks x (3 min, 3 max); host combines."""
    from concourse import bass, mybir

    nc = bass.Bass(trn_type="TRN2")
    dt = mybir.dt
    op = mybir.AluOpType

    pts_ext = nc.declare_dram_parameter("pts", [128, 3 * W], dt.float32, isOutput=False)
    mm_ext = nc.declare_dram_parameter("mm", [128, 24], dt.float32, isOutput=True)

    cuts = [0, W // 4, W // 2, 3 * W // 4, W]
    chunks = list(zip(cuts[:-1], cuts[1:]))

    with (
        nc.Block() as block,
        nc.semaphore("dma") as dma,
        nc.semaphore("cmp") as cmp,
        nc.sbuf_tensor("xyz", [128, 3 * W], dt.float32) as xyz,
        nc.sbuf_tensor("mm_sb", [128, 24], dt.float32) as mm,
    ):
        @block.sync
        def _(sy):
            for a, b in chunks:
                sy.dma_start(
                    out=xyz[:, 3 * a : 3 * b], in_=pts_ext[:, 3 * a : 3 * b]
                ).then_inc(dma, 16)
            sy.wait_ge(cmp, 4)
            sy.dma_start(out=mm_ext[:, :], in_=mm[:, :]).then_inc(dma, 16)
            sy.wait_ge(dma, 80)

        @block.vector
        def _(v):
            for k, (a, b) in enumerate(chunks):
                v.wait_ge(dma, 16 * (k + 1))
                for c in range(3):
                    xv = xyz[:, 3 * a + c : 3 * b : 3]
                    o = 6 * k
                    v.tensor_reduce(
                        mm[:, o + c : o + c + 1], xv, mybir.AxisListType.XYZW, op.min
                    )
                    ins = v.tensor_reduce(
                        mm[:, o + 3 + c : o + 4 + c], xv, mybir.AxisListType.XYZW,
                        op.max,
                    )
                    if c == 2:
                        ins.then_inc(cmp, 1)

    return nc


def build_graph_coord(W, out_dt_name="int16"):
    """Per-point grid coords + suspect flag.

    Chunk-pipelined over two engines: ACT computes d = x-mn, q0 = d*(1/v),
    r = round_to_i16(q0) for chunk k while DVE runs the floor fix-up
    (fl = r - (r > q0)), the product-form boundary-suspect test
    (m = fr^2 - fr is near 0 iff fr is near 0 or 1), and the flag pack on
    chunk k-1.  gi1/gi2 DMA out as soon as written; gi0 after the pack.
    (GPSIMD was evaluated as a third worker but the Pool engine lacks
    comparison ALU ops.)

    scal columns: 0..2 = -mins(xyz), 3 = 1/v, 4 = -T2 (negated, 2x-inflated
    suspect threshold 2^-20*qmax), 5..7 unused.
    """
    from concourse import bass, mybir

    nc = bass.Bass(trn_type="TRN2")
    dt = mybir.dt
    op = mybir.AluOpType
    af = mybir.ActivationFunctionType
    odt = getattr(dt, out_dt_name)
    packc = -32768.0 if out_dt_name == "int16" else -float(2**30)

    pts_ext = nc.declare_dram_parameter("pts", [128, 3 * W], dt.float32, isOutput=False)
    scal_ext = nc.declare_dram_parameter("scal", [128, 8], dt.float32, isOutput=False)
    g_ext = [
        nc.declare_dram_parameter(f"g{c}", [128, W], odt, isOutput=True)
        for c in range(3)
    ]

    # eighths; all chunks on the vector engine (Pool lacks compare ops)
    fr8 = [0, 1, 3, 6, 8]
    owners = ["D", "D", "D", "D"]
    cuts = [W * f // 8 for f in fr8]
    chunks = list(zip(cuts[:-1], cuts[1:]))
    NCH = len(chunks)

    ctx = []

    def sb(name, shape, dtype):
        cm = nc.sbuf_tensor(name, shape, dtype)
        h = cm.__enter__()
        ctx.append(cm)
        return h

    Wq = max(b - a for a, b in chunks)
    xyz = sb("xyz", [128, 3 * W], dt.float32)
    scal = sb("scal_sb", [128, 8], dt.float32)
    bd = sb("bd", [128, Wq], dt.float32)
    q0 = [sb(f"q0_{c}", [128, W], dt.float32) for c in range(3)]
    rr = [sb(f"rr_{c}", [128, W], dt.int16) for c in range(3)]
    gi = [sb(f"gi{c}", [128, W], odt) for c in range(3)]
    # chunk-local temp sets per worker engine
    tmp = {}
    for e in ("D",):
        tmp[e] = dict(
            bC=sb(f"bC{e}", [128, Wq], dt.float32),
            bD=sb(f"bD{e}", [128, Wq], dt.float32),
            ss=[sb(f"ss{e}{c}", [128, Wq], dt.float32) for c in range(3)],
            fl0=sb(f"fl0{e}", [128, Wq], odt),
        )

    def worker_body(v, e, done_sem, cmpA):
        t = tmp[e]
        bC, bD, ss, fl0 = (t["bC"], t["bD"], t["ss"], t["fl0"])
        mt2 = scal[:, 4:5]
        v.wait_ge(cmpA, 1)
        for k, (a, b) in enumerate(chunks):
            if owners[k] != e:
                continue
            s = slice(a, b)
            q = slice(0, b - a)
            if True:
                # DVE: mixed int16/f32 allowed; fl lands in int16 outputs
                for c in range(3):
                    dst, ds = (fl0, q) if c == 0 else (gi[c], s)
                    v.wait_ge(cmpA, 2 + 3 * k + c)
                    v.tensor_tensor(bC[:, q], rr[c][:, s], q0[c][:, s], op.is_gt)
                    ins = v.tensor_tensor(dst[:, ds], rr[c][:, s], bC[:, q], op.subtract)
                    if c == 2:
                        ins.then_inc(cmpE, 1)
                    # fr = q0 - fl (exact); m~ = fr^2 - fr is near 0 iff fr
                    # is near 0 or 1 (the suspect zone)
                    v.tensor_tensor(bD[:, q], q0[c][:, s], dst[:, ds], op.subtract)
                    v.scalar_tensor_tensor(
                        ss[c][:, q], bD[:, q], 1.0, bD[:, q], op.subtract, op.mult
                    )
                v.tensor_tensor(bC[:, q], ss[0][:, q], ss[1][:, q], op.max)
                v.tensor_tensor(bD[:, q], bC[:, q], ss[2][:, q], op.max)
                v.tensor_scalar(bC[:, q], bD[:, q], mt2, packc, op.is_gt, op.mult)
                v.tensor_tensor(gi[0][:, s], fl0[:, q], bC[:, q], op.add).then_inc(
                    done_sem, 1
                )

    with (
        nc.Block() as block,
        nc.semaphore("din") as din,
        nc.semaphore("dina") as dina,
        nc.semaphore("dout") as dout,
        nc.semaphore("cmpA") as cmpA,
        nc.semaphore("cmpD") as cmpD,
        nc.semaphore("cmpE") as cmpE,
    ):
        @block.sync
        def _(sy):
            for a, b in chunks:
                sy.dma_start(
                    out=xyz[:, 3 * a : 3 * b], in_=pts_ext[:, 3 * a : 3 * b]
                ).then_inc(din, 16)
            for k in range(NCH):
                a, b = chunks[k]
                sy.wait_ge(cmpE, k + 1)
                for c in (1, 2):
                    sy.dma_start(
                        out=g_ext[c][:, a:b], in_=gi[c][:, a:b]
                    ).then_inc(dout, 16)
                sy.wait_ge(cmpD, k + 1)
                sy.dma_start(
                    out=g_ext[0][:, a:b], in_=gi[0][:, a:b]
                ).then_inc(dout, 16)
            sy.wait_ge(dout, NCH * 48)

        @block.scalar
        def _(sc):
            negmn = [scal[:, c : c + 1] for c in range(3)]
            rc = scal[:, 3:4]
            sc.dma_start(out=scal[:, :], in_=scal_ext[:, :]).then_inc(dina, 16)
            sc.activation(bd[:, 0:4], bd[:, 0:4], af.Copy).then_inc(cmpA, 1)
            sc.wait_ge(dina, 16)
            for k, (a, b) in enumerate(chunks):
                sc.wait_ge(din, 16 + 16 * k)
                s = slice(a, b)
                q = slice(0, b - a)
                for c in range(3):
                    xv = xyz[:, 3 * a + c : 3 * b : 3]
                    sc.activation(bd[:, q], xv, af.Identity, bias=negmn[c], scale=1.0)
                    sc.activation(q0[c][:, s], bd[:, q], af.Copy, bias=0.0, scale=rc)
                    sc.activation(rr[c][:, s], q0[c][:, s], af.Copy).then_inc(
                        cmpA, 1
                    )

        @block.vector
        def _(v):
            worker_body(v, "D", cmpD, cmpA)

    for cm in reversed(ctx):
        cm.__exit__(None, None, None)
    return nc


# ----------------------------------------------------------------------------
# compile-once SPMD runner (mirrors bass2jax.run_bass_via_pjrt)
# ----------------------------------------------------------------------------

class SpmdRunner:
    def __init__(self, nc, n_cores=N_CORES):
        import jax
        from concourse import mybir
        from concourse.bass2jax import (
            _bass_exec_p,
            install_neuronx_cc_hook,
            partition_id_tensor,
        )
        from jax.sharding import Mesh, PartitionSpec
        try:
            from jax.experimental.shard_map import shard_map
        except ImportError:
            from jax.shard_map import shard_map

        install_neuronx_cc_hook()
        assert not (nc.dbg_addr is not None and nc.dbg_callbacks)
        self.nc = nc
        self.n_cores = n_cores
        partition_name = nc.partition_id_tensor.name if nc.partition_id_tensor else None

        in_names, out_names, out_avals, zero_shapes = [], [], [], []
        for alloc in nc.m.functions[0].allocations:
            if not isinstance(alloc, mybir.MemoryLocationSet):
                continue
            name = alloc.memorylocations[0].name
            if alloc.kind == "ExternalInput":
                if name != partition_name and name != (
                    nc.dbg_addr.name if nc.dbg_addr is not None else None
                ):
                    in_names.append(name)
            elif alloc.kind == "ExternalOutput":
                shape = tuple(alloc.tensor_shape)
                dtype = mybir.dt.np(alloc.dtype)
                out_names.append(name)
                out_avals.append(jax.core.ShapedArray(shape, dtype))
                zero_shapes.append((shape, dtype))
        self.has_dbg = nc.dbg_addr is not None
        if self.has_dbg:
            in_names.append(nc.dbg_addr.name)
        n_params = len(in_names)
        self.in_names = list(in_names)
        self.out_names = out_names
        self.out_avals = out_avals
        self.zero_shapes = zero_shapes
        self.n_params = n_params

        all_in_names = in_names + out_names
        if partition_name is not None:
            all_in_names.append(partition_name)
        donate = tuple(range(n_params, n_params + len(out_names)))

        def _body(*args):
            operands = list(args)
            if partition_name is not None:
                operands.append(partition_id_tensor())
            outs = _bass_exec_p.bind(
                *operands,
                out_avals=tuple(out_avals),
                in_names=tuple(all_in_names),
                out_names=tuple(out_names),
                lowering_input_output_aliases=(),
                sim_require_finite=True,
                sim_require_nnan=True,
                nc=nc,
            )
            return tuple(outs)

        devices = jax.devices()[:n_cores]
        mesh = Mesh(np.asarray(devices), ("core",))
        in_specs = (PartitionSpec("core"),) * (n_params + len(out_names))
        out_specs = (PartitionSpec("core"),) * len(out_names)
        self._fn = jax.jit(
            shard_map(
                _body,
                mesh=mesh,
                in_specs=in_specs,
                out_specs=out_specs,
                check_rep=False,
            ),
            donate_argnums=donate,
            keep_unused=True,
        )

    def run(self, in_maps):
        per_core = []
        for m in in_maps:
            m = dict(m)
            if self.has_dbg:
                m[self.nc.dbg_addr.name] = np.zeros((1, 2), np.uint32)
            per_core.append([np.ascontiguousarray(m[k]) for k in self.in_names])
        concat_in = [
            np.concatenate([per_core[c][i] for c in range(self.n_cores)], axis=0)
            for i in range(self.n_params)
        ]
        outs = None
        for attempt in range(3):
            concat_zeros = [
                np.zeros((self.n_cores * s[0], *s[1:]), d)
                for (s, d) in self.zero_shapes
            ]
            try:
                outs = self._fn(*concat_in, *concat_zeros)
                break
            except Exception:
                if attempt == 2:
                    raise
                # transient NRT device errors: reconnect the backend and
                # rebuild the jitted executable (compile cache makes this
                # cheap), then retry
                import time

                time.sleep(10)
                try:
                    import jax

                    jax.clear_backends()
                except Exception:
                    pass
                self.__init__(self.nc, self.n_cores)
        return [
            {
                name: np.asarray(outs[i]).reshape(
                    self.n_cores, *self.out_avals[i].shape
                )[c]
                for i, name in enumerate(self.out_names)
            }
            for c in range(self.n_cores)
        ]


# ----------------------------------------------------------------------------
# host orchestration
# ----------------------------------------------------------------------------

def _layout(comp_lid):
    """Shard points by component; pad each component to a multiple of the
    per-partition width W so each SBUF partition is single-component."""
    bounds = np.searchsorted(comp_lid, np.arange(G + 1)).astype(np.int64)
    lens = np.diff(bounds)
    core_pts = [
        int(bounds[(c + 1) * COMPS_PER_CORE] - bounds[c * COMPS_PER_CORE])
        for c in range(N_CORES)
    ]
    W = -(-max(core_pts) // 120)
    W = (W + 3) & ~3
    while True:
        ok = all(
            sum(
                -(-int(lens[g]) // W)
                for g in range(c * COMPS_PER_CORE, (c + 1) * COMPS_PER_CORE)
            )
            <= 128
            for c in range(N_CORES)
        )
        if ok:
            break
        W += 64
    slots = {}  # comp -> (core, part_start, n_parts, length)
    part2comp = np.zeros((N_CORES, 128), np.int64)
    for c in range(N_CORES):
        p = 0
        for g in range(c * COMPS_PER_CORE, (c + 1) * COMPS_PER_CORE):
            L = int(lens[g])
            k = -(-L // W)
            slots[g] = (c, p, k, L)
            part2comp[c, p : p + k] = g
            p += k
        part2comp[c, p:] = (c + 1) * COMPS_PER_CORE - 1
    return bounds, lens, W, slots, part2comp


def _pack_points(p_xyz, bounds, W, slots):
    pts = np.zeros((N_CORES, 128 * W, 3), np.float32)
    for g, (c, p, k, L) in slots.items():
        seg = p_xyz[bounds[g] : bounds[g + 1]]
        blk = pts[c, p * W : (p + k) * W]
        blk[:L] = seg
        blk[L:] = seg[0]
        if p + k < 128 and g % COMPS_PER_CORE == COMPS_PER_CORE - 1:
            pts[c, (p + k) * W :] = seg[0]
    return [pts[c].reshape(128, 3 * W) for c in range(N_CORES)]


def _unpack(outs, key, bounds, W, slots, dtype):
    full = np.empty(N, dtype)
    for g, (c, p, k, L) in slots.items():
        flat = outs[c][key].reshape(-1)
        full[bounds[g] : bounds[g + 1]] = flat[p * W : p * W + L]
    return full


def _grid_exact_host(x, mn, v):
    """floor of correctly-rounded f32 division, matching jax CPU."""
    d = (x.astype(np.float32) - mn.astype(np.float32)).astype(np.float32)
    q = (d / v.astype(np.float32)).astype(np.float32)
    return np.floor(q).astype(np.int64)


def kernel(p_xyz, comp_lid, s_alloc):
    p_xyz = np.asarray(p_xyz, np.float32)
    comp_lid = np.asarray(comp_lid, np.int64)
    s_alloc = np.asarray(s_alloc, np.int64)

    bounds, lens, W, slots, part2comp = _layout(comp_lid)
    pts_maps = _pack_points(p_xyz, bounds, W, slots)

    nc_bbox = build_graph_bbox(W)
    runner_bbox = SpmdRunner(nc_bbox)
    nc_coord = build_graph_coord(W)
    runner_coord = SpmdRunner(nc_coord)
    LAST_DEBUG.clear()
    LAST_DEBUG.update(
        nc_bbox=nc_bbox, nc_coord=nc_coord, W=W, n_exec=0, passes=[]
    )
    state = {"runner32": None}

    # ---- pass 0: bbox ----
    outs0 = runner_bbox.run([{"pts": pts_maps[c]} for c in range(N_CORES)])
    LAST_DEBUG["n_exec"] += 1
    mins = np.full((G, 3), np.inf, np.float32)
    maxs = np.full((G, 3), -np.inf, np.float32)
    for g, (c, p, k, L) in slots.items():
        rows = outs0[c]["mm"][p : p + k]
        rmin = rows[:, 0:3]
        rmax = rows[:, 3:6]
        for o in range(6, 24, 6):
            rmin = np.minimum(rmin, rows[:, o : o + 3])
            rmax = np.maximum(rmax, rows[:, o + 3 : o + 6])
        mins[g] = rmin.min(axis=0)
        maxs[g] = rmax.max(axis=0)
    LAST_DEBUG["mins"] = mins.copy()
    LAST_DEBUG["maxs"] = maxs.copy()

    f32 = np.float32
    span_true = (maxs - mins).astype(f32)  # for qmax bound

    def exec_pass(v_f32):
        """One coord pass; returns exact per-point int64 grids [N,3]."""
        qmax = span_true.max(axis=1) / v_f32 + 2.0  # float64 upper bound
        use16 = bool(np.all(qmax < 32000.0))
        if use16:
            runner = runner_coord
        else:
            if state["runner32"] is None:
                nc32 = build_graph_coord(W, "int32")
                state["runner32"] = SpmdRunner(nc32)
            runner = state["runner32"]
        scal_maps = []
        for c in range(N_CORES):
            sc = np.zeros((128, 8), np.float32)
            gidx = part2comp[c]
            sc[:, 0:3] = -mins[gidx]
            sc[:, 3] = (f32(1.0) / v_f32[gidx]).astype(f32)
            tp = (qmax[gidx] * (2.0**-20)).astype(f32)  # 2x-inflated threshold
            sc[:, 4] = -tp
            scal_maps.append(sc)
        outs = runner.run(
            [{"pts": pts_maps[c], "scal": scal_maps[c]} for c in range(N_CORES)]
        )
        LAST_DEBUG["n_exec"] += 1
        grids = np.empty((N, 3), np.int64)
        packc = 32768 if use16 else 2**30
        p0 = _unpack(outs, "g0", bounds, W, slots, np.int64)
        sus_mask = p0 < 0
        grids[:, 0] = np.where(sus_mask, p0 + packc, p0)
        grids[:, 1] = _unpack(outs, "g1", bounds, W, slots, np.int64)
        grids[:, 2] = _unpack(outs, "g2", bounds, W, slots, np.int64)
        sus = np.nonzero(sus_mask)[0]
        if sus.size:
            comp_s = comp_lid[sus]
            for axis in range(3):
                grids[sus, axis] = _grid_exact_host(
                    p_xyz[sus, axis], mins[comp_s, axis], v_f32[comp_s]
                )
        LAST_DEBUG["passes"].append(dict(v=v_f32.copy(), n_suspect=int(sus.size)))
        return grids

    # ---- host O(G) setup (mirrors reference f32/int64 semantics) ----
    span = np.maximum(span_true, f32(1e-6))
    safe_span = np.maximum(span, f32(0.05))
    safe_vol = (safe_span[:, 0] * safe_span[:, 1] * safe_span[:, 2]).astype(f32)
    n_per = lens.astype(np.int64)
    s_c = np.clip(s_alloc, 1, np.maximum(n_per, np.int64(1)))
    v0 = np.power(
        (safe_vol / np.maximum(s_c.astype(f32), f32(1.0))).astype(f32), 1.0 / 3.0
    ).astype(f32)
    v_lo = np.maximum((v0 * f32(0.1)).astype(f32), f32(1e-4))
    v_hi = np.maximum((v0 * f32(64.0)).astype(f32), f32(1e-4))
    best_v = v0.copy()
    best_diff = np.full(G, 1 << 30, np.int64)

    def distinct_counts(grids):
        h = (
            (grids[:, 0] * np.int64(P1))
            ^ (grids[:, 1] * np.int64(P2))
            ^ (grids[:, 2] * np.int64(P3))
        ) & np.int64(MASK40)
        cnt = np.empty(G, np.int64)
        for g in range(G):
            cnt[g] = np.unique(h[bounds[g] : bounds[g + 1]]).size
        return cnt, h

    # ---- bisection ----
    # best_v is always one of the visited v_mids (first iteration always
    # improves the 1<<30 sentinel), so cache per-pass grids and skip the
    # final voxelization pass entirely.
    pass_grids = []
    best_pass = np.zeros(G, np.int64)
    for it in range(N_BISECT):
        v_mid = ((v_lo + v_hi) * f32(0.5)).astype(f32)
        grids = exec_pass(v_mid)
        pass_grids.append(grids.astype(np.int32))
        cnt, _ = distinct_counts(grids)
        curr_diff = np.abs(cnt - s_c)
        improved = curr_diff < best_diff
        best_diff = np.where(improved, curr_diff, best_diff)
        best_v = np.where(improved, v_mid, best_v).astype(f32)
        best_pass = np.where(improved, it, best_pass)
        more = cnt > s_c
        v_lo = np.where(more, v_mid, v_lo).astype(f32)
        v_hi = np.where(more, v_hi, v_mid).astype(f32)
        LAST_DEBUG["passes"][-1]["cnt"] = cnt.copy()

    # ---- final voxelization: reuse the winning pass's grids per component
    grids = np.empty((N, 3), np.int64)
    for g in range(G):
        sl = slice(bounds[g], bounds[g + 1])
        grids[sl] = pass_grids[int(best_pass[g])][sl]
    _, h = distinct_counts(grids)
    keys = (comp_lid << np.int64(40)) | h

    uniq, inv = np.unique(keys, return_inverse=True)
    U = uniq.size
    inv = inv.astype(np.int64)

    counts = np.bincount(inv, minlength=U).astype(np.int64)
    sums = np.zeros((U, 3), np.float32)
    np.add.at(sums, inv, p_xyz)
    cent = (sums / np.maximum(counts, 1).astype(np.float32)[:, None]).astype(
        np.float32
    )
    diff = (p_xyz - cent[inv]).astype(np.float32)
    d = (
        (diff[:, 0] * diff[:, 0] + diff[:, 1] * diff[:, 1]) + diff[:, 2] * diff[:, 2]
    ).astype(np.float32)

    order = np.argsort(inv, kind="stable")
    starts = np.searchsorted(inv[order], np.arange(U))
    dmin = np.minimum.reduceat(d[order], starts)
    pos = np.arange(N, dtype=np.int64)
    cand = np.where(d <= dmin[inv], pos, np.int64(N))
    argmin = np.minimum.reduceat(cand[order], starts)

    seeds_xyz = np.zeros((N, 3), np.float32)
    seeds_gid = np.full(N, -1, np.int64)
    valid = np.zeros(N, np.bool_)
    idx = np.clip(argmin, 0, N - 1)
    seeds_xyz[:U] = p_xyz[idx]
    seeds_gid[:U] = idx
    valid[:U] = True
    LAST_DEBUG["U"] = U
    return seeds_xyz, seeds_gid, valid
